# revision 10
# baseline (speedup 1.0000x reference)
"""EnhancedCGConv GNN message-passing kernel for 8 Trainium2 NeuronCores.

Strategy (dst-sharded, zero collectives):
  - Edges are grouped on the host by destination node; node range [c*N/8,
    (c+1)*N/8) is owned by core c.  Within a core, edges are grouped by
    128-node "node tile" and padded to L=18 edge tiles of 128 slots each, so
    every core runs the IDENTICAL static program (SPMD, compile once).
  - Per edge tile the device computes the edge MLP, gathers source-node
    features (indirect DMA), applies the K message kernels, multiplies by the
    edge weights, and scatter-adds into the node-tile accumulator via a
    one-hot matmul (onehot[e, n] = (dst_rel[e] == n)) on the TensorEngine.
  - Per node tile: mean (host-computed 1/count), fusion dense via PE
    transposes, group norm (16 groups of 4), gamma/beta, relu, store.
  - Matmuls run in float32r (TF32 path, 1 cycle/row at free-dim >= 256);
    fp32 accumulation in PSUM.  The fusion stage runs in plain fp32.
"""

import os
import sys

import numpy as np

for _p in ("/opt/trn_rl_repo", "/root/.axon_site/_ro/trn_rl_repo"):
    if os.path.isdir(_p) and _p not in sys.path:
        sys.path.insert(0, _p)

from concourse import bacc, bass, library_config, mybir, tile  # noqa: E402

P = 128
F = 64          # node feature dim
U = 64          # units
KK = 3          # num kernels
KU = KK * U     # 192
KUP = 256       # padded to 256 to trigger the fast f32r matmul path
G = 16          # group-norm groups
EPS = 1e-5

f32 = mybir.dt.float32
f32r = mybir.dt.float32r
i32 = mybir.dt.int32


class Cfg:
    def __init__(self, N, E, ncores, L):
        self.N = N
        self.E = E
        self.ncores = ncores
        assert N % ncores == 0
        self.NPC = N // ncores           # nodes per core
        self.NT = (self.NPC + P - 1) // P  # node tiles per core
        self.L = L                        # edge tiles per node tile
        self.S = self.NT * self.L * P     # edge slots per core


FULL = Cfg(N=50000, E=800000, ncores=8, L=18)


# ---------------------------------------------------------------- host prep

def prepare_inputs(cfg, inputs):
    """Shard + slot edges; returns (in_maps, None) for run_bass_kernel_spmd."""
    nf = np.ascontiguousarray(np.asarray(inputs["node_features"], np.float32))
    ei = np.asarray(inputs["edge_indices"])
    src = ei[0].astype(np.int64)
    dst = ei[1].astype(np.int64)
    ef = np.ascontiguousarray(np.asarray(inputs["edge_features"], np.float32))
    W1 = np.asarray(inputs["edge_W1"], np.float32)
    b1 = np.asarray(inputs["edge_b1"], np.float32)
    W2 = np.asarray(inputs["edge_W2"], np.float32)
    b2 = np.asarray(inputs["edge_b2"], np.float32)
    Wk = np.asarray(inputs["W_kernels"], np.float32)
    fW = np.asarray(inputs["fusion_W"], np.float32)
    fb = np.asarray(inputs["fusion_b"], np.float32)
    gamma = np.asarray(inputs["gamma"], np.float32)
    beta = np.asarray(inputs["beta"], np.float32)

    N, E, NPC, NT, L, S = cfg.N, cfg.E, cfg.NPC, cfg.NT, cfg.L, cfg.S
    ncores = cfg.ncores

    core = dst // NPC
    loc = dst - core * NPC
    t = loc >> 7
    gt = core * NT + t
    cnt_gt = np.bincount(gt, minlength=ncores * NT)
    if cnt_gt.max() > L * P:
        raise OverflowError(int(np.ceil(cnt_gt.max() / P)))

    order = np.argsort(gt, kind="stable")
    gts = gt[order]
    starts = np.zeros(ncores * NT + 1, np.int64)
    np.cumsum(cnt_gt, out=starts[1:])
    rank = np.arange(E, dtype=np.int64) - starts[gts]
    slot = gts * (L * P) + rank           # global slot of edge order[i]

    TOT = ncores * S
    ef_s = np.zeros((TOT, F), np.float32)
    ef_s[slot] = ef[order]
    pidx_s = np.zeros(TOT, np.int16)
    pidx_s[slot] = (src[order] >> 1).astype(np.int16)
    par_s = np.zeros(TOT, np.float32)
    par_s[slot] = (src[order] & 1).astype(np.float32)
    dstv_s = np.full(TOT, -1.0, np.float32)
    dstv_s[slot] = (loc - (t << 7))[order].astype(np.float32)

    cntN = np.bincount(dst, minlength=N).astype(np.float32)
    recip = (1.0 / np.maximum(cntN, 1.0)).astype(np.float32)

    W_all = np.ascontiguousarray(Wk.transpose(1, 0, 2).reshape(F, KU))
    W1a = np.zeros((F, 65), np.float32)
    W1a[:, :U] = W1
    b1a = np.zeros((65, 1), np.float32)
    b1a[:U, 0] = b1
    b1a[U, 0] = 1.0
    W2cat = np.zeros((65, KUP), np.float32)
    W2cat[:U, :KU] = W2
    W2cat[U, :KU] = b2
    Wallp = np.zeros((2 * F, KUP), np.float32)
    Wallp[:F, :KU] = W_all
    Wallp[F:, :KU] = W_all
    fWa = np.ascontiguousarray(fW[:96])
    fWb = np.ascontiguousarray(fW[96:])
    fbrow = np.ascontiguousarray(fb[None, :])
    iota = np.broadcast_to(np.arange(P, dtype=np.float32), (P, P)).copy()
    ident = np.eye(P, dtype=np.float32)

    def tf32(x):
        u = np.ascontiguousarray(x, np.float32).view(np.uint32)
        return ((u + 0x1000) & 0xFFFFE000).view(np.float32)

    assert N % 2 == 0
    nfp = np.ascontiguousarray(tf32(nf).reshape(N // 2, 2 * F))
    W1a = tf32(W1a)
    W2cat = tf32(W2cat)
    Wallp = tf32(Wallp)

    shared = {
        "nfp": nfp,
        "w1a": W1a, "b1a": b1a, "w2cat": W2cat, "wallp": Wallp,
        "fwa": fWa, "fwb": fWb, "fbrow": fbrow,
        "gamma": np.ascontiguousarray(gamma[None, :]),
        "beta": np.ascontiguousarray(beta[None, :]),
        "iota": iota, "ident": ident,
    }

    in_maps = []
    for c in range(ncores):
        sl = slice(c * S, (c + 1) * S)
        efT_c = tf32(np.ascontiguousarray(ef_s[sl].T))
        # dma_gather index layout: [16 partitions, num_idxs/16] per node tile,
        # flat slot i = s*16 + p; rows 16..127 must hold in-range values.
        SI = L * P // 16
        idx16_c = np.ascontiguousarray(np.tile(
            pidx_s[sl].reshape(NT, SI, 16).transpose(2, 0, 1)
            .reshape(16, NT * SI), (8, 1)))
        par_t = par_s[sl].reshape(NT, L, P).transpose(2, 0, 1).reshape(P, NT * L)
        parv_c = np.ascontiguousarray(par_t)
        qarv_c = np.ascontiguousarray(1.0 - par_t)
        dstv_c = np.ascontiguousarray(
            dstv_s[sl].reshape(NT, L, P).transpose(2, 0, 1).reshape(P, NT * L))
        rp = np.ones(NT * P, np.float32)
        rp[:NPC] = recip[c * NPC:(c + 1) * NPC]
        recip_c = np.ascontiguousarray(rp.reshape(NT, P).T)
        m = dict(shared)
        m.update({"eft": efT_c, "idx16": idx16_c, "dstv": dstv_c,
                  "parv": parv_c, "qarv": qarv_c, "recip": recip_c})
        in_maps.append(m)
    return in_maps


# ---------------------------------------------------------------- device IR

def build_nc(cfg):
    N, NPC, NT, L = cfg.N, cfg.NPC, cfg.NT, cfg.L
    S = cfg.S
    nc = bacc.Bacc("TRN2", target_bir_lowering=False)

    d_eft = nc.dram_tensor("eft", [F, S], f32, kind="ExternalInput")
    SI = L * P // 16
    i16 = mybir.dt.int16
    d_idx16 = nc.dram_tensor("idx16", [P, NT * SI], i16, kind="ExternalInput")
    d_dstv = nc.dram_tensor("dstv", [P, NT * L], f32, kind="ExternalInput")
    d_parv = nc.dram_tensor("parv", [P, NT * L], f32, kind="ExternalInput")
    d_qarv = nc.dram_tensor("qarv", [P, NT * L], f32, kind="ExternalInput")
    d_recip = nc.dram_tensor("recip", [P, NT], f32, kind="ExternalInput")
    d_nfp = nc.dram_tensor("nfp", [N // 2, 2 * F], f32, kind="ExternalInput")
    d_w1a = nc.dram_tensor("w1a", [F, 65], f32, kind="ExternalInput")
    d_b1a = nc.dram_tensor("b1a", [65, 1], f32, kind="ExternalInput")
    d_w2cat = nc.dram_tensor("w2cat", [65, KUP], f32, kind="ExternalInput")
    d_wallp = nc.dram_tensor("wallp", [2 * F, KUP], f32, kind="ExternalInput")
    d_fwa = nc.dram_tensor("fwa", [96, U], f32, kind="ExternalInput")
    d_fwb = nc.dram_tensor("fwb", [96, U], f32, kind="ExternalInput")
    d_fbrow = nc.dram_tensor("fbrow", [1, U], f32, kind="ExternalInput")
    d_gamma = nc.dram_tensor("gamma", [1, U], f32, kind="ExternalInput")
    d_beta = nc.dram_tensor("beta", [1, U], f32, kind="ExternalInput")
    d_out = nc.dram_tensor("out", [NPC, U], f32, kind="ExternalOutput")

    ACT = mybir.ActivationFunctionType
    ALU = mybir.AluOpType
    AX = mybir.AxisListType

    # mm1 processes edge slots in chunks of up to 512 columns
    CH = 512
    n_chunks = (L * P + CH - 1) // CH

    with tile.TileContext(nc) as tc:
        with tc.tile_pool(name="const", bufs=1) as const, \
             tc.tile_pool(name="efp", bufs=2) as efp, \
             tc.tile_pool(name="gp", bufs=2) as gp, \
             tc.tile_pool(name="hp", bufs=2) as hp, \
             tc.tile_pool(name="srcp", bufs=3) as srcp, \
             tc.tile_pool(name="msgp", bufs=3) as msgp, \
             tc.tile_pool(name="ohp", bufs=3) as ohp, \
             tc.tile_pool(name="epi", bufs=2) as epi, \
             tc.tile_pool(name="ph", bufs=2, space="PSUM") as ph, \
             tc.tile_pool(name="pem", bufs=2, space="PSUM") as pem, \
             tc.tile_pool(name="ptr", bufs=1, space="PSUM") as ptr, \
             tc.tile_pool(name="pacc", bufs=2, space="PSUM") as pacc:

            nc.gpsimd.load_library(library_config.mlp)

            def load_const(name, dram, shape, dtype=f32r):
                t_ = const.tile(shape, dtype, tag=name)
                nc.sync.dma_start(out=t_[:], in_=dram[:].bitcast(dtype))
                return t_

            w1a = load_const("w1a", d_w1a, [F, 65])
            b1a = load_const("b1a", d_b1a, [65, 1], f32)
            w2cat = load_const("w2cat", d_w2cat, [65, KUP])
            wallp = load_const("wallp", d_wallp, [2 * F, KUP])
            fwa = load_const("fwa", d_fwa, [96, U], f32)
            fwb = load_const("fwb", d_fwb, [96, U], f32)
            fbrow = load_const("fbrow", d_fbrow, [1, U], f32)
            ones1 = const.tile([1, P], f32)
            nc.vector.memset(ones1[:], 1.0)
            epsb = const.tile([P, 1], f32)
            nc.vector.memset(epsb[:], EPS)

            d_iota = nc.dram_tensor("iota", [P, P], f32, kind="ExternalInput")
            iota_r = const.tile([P, P], f32)
            nc.sync.dma_start(out=iota_r[:], in_=d_iota[:])
            d_ident = nc.dram_tensor("ident", [P, P], f32, kind="ExternalInput")
            ident_r = const.tile([P, P], f32r)
            nc.sync.dma_start(out=ident_r[:], in_=d_ident[:].bitcast(f32r))
            ident_f = const.tile([P, P], f32)
            nc.sync.dma_start(out=ident_f[:], in_=d_ident[:])

            # broadcast gamma/beta across partitions
            gammab = const.tile([P, U], f32)
            nc.gpsimd.dma_start(
                out=gammab[:],
                in_=bass.AP(tensor=d_gamma, offset=0, ap=[[0, P], [1, U]]))
            betab = const.tile([P, U], f32)
            nc.gpsimd.dma_start(
                out=betab[:],
                in_=bass.AP(tensor=d_beta, offset=0, ap=[[0, P], [1, U]]))

            idx_all = const.tile([P, NT * SI], i16)
            nc.sync.dma_start(out=idx_all[:], in_=d_idx16[:])
            dstv_all = const.tile([P, NT * L], f32)
            nc.sync.dma_start(out=dstv_all[:], in_=d_dstv[:])
            parv_all = const.tile([P, NT * L], f32)
            nc.sync.dma_start(out=parv_all[:], in_=d_parv[:])
            qarv_all = const.tile([P, NT * L], f32)
            nc.sync.dma_start(out=qarv_all[:], in_=d_qarv[:])
            recip_all = const.tile([P, NT], f32)
            nc.sync.dma_start(out=recip_all[:], in_=d_recip[:])

            for t in range(NT):
                rows = min(P, NPC - t * P)
                # stage this node tile's edge-feature slots and gather indices
                efc = efp.tile([F, L * P], f32r)
                nc.sync.dma_start(
                    out=efc[:], in_=d_eft[:, t * L * P:(t + 1) * L * P].bitcast(f32r))
                gath = gp.tile([P, L, 2 * F], f32)
                nc.gpsimd.dma_gather(
                    out_ap=gath[:], in_ap=d_nfp[:],
                    idxs_ap=idx_all[:, t * SI:(t + 1) * SI],
                    num_idxs=L * P, num_idxs_reg=L * P, elem_size=2 * F,
                    single_packet=False)

                acc = pacc.tile([P, KUP], f32, space="PSUM")
                hT = None
                for j in range(L):
                    cidx, coff = divmod(j * P, CH)
                    if coff == 0:
                        cw = min(CH, L * P - cidx * CH)
                        hps = ph.tile([65, CH], f32, space="PSUM")
                        nc.tensor.matmul(
                            out=hps[:, :cw], lhsT=w1a[:],
                            rhs=efc[:, cidx * CH: cidx * CH + cw],
                            start=True, stop=True)
                        hT = hp.tile([65, CH], f32r)
                        nc.scalar.activation(
                            out=hT[:, :cw], in_=hps[:, :cw], func=ACT.Relu,
                            bias=b1a[:, 0:1], scale=1.0)
                    # edge MLP layer 2 (+b2 via the ones row of hT)
                    ewml = pem.tile([P, 2 * KUP], f32, space="PSUM")
                    nc.tensor.matmul(
                        out=ewml[:, :KUP], lhsT=hT[:, coff:coff + P],
                        rhs=w2cat[:], start=True, stop=True)
                    # parity-select pair halves (GpSimd), then PE transpose
                    sc = t * L + j
                    gsel = gp.tile([P, 2 * F], f32, tag="gsel")
                    nc.gpsimd.tensor_scalar_mul(
                        out=gsel[:, :F], in0=gath[:, j, :F],
                        scalar1=qarv_all[:, sc:sc + 1])
                    nc.gpsimd.tensor_scalar_mul(
                        out=gsel[:, F:], in0=gath[:, j, F:],
                        scalar1=parv_all[:, sc:sc + 1])
                    trps = ptr.tile([2 * F, P], f32, space="PSUM", tag="trps")
                    nc.tensor.transpose(
                        out=trps[:], in_=gsel[:], identity=ident_f[:])
                    srcT = srcp.tile([2 * F, P], f32r)
                    nc.scalar.activation(
                        out=srcT[:], in_=trps[:], func=ACT.Copy)
                    # message linear part (both halves summed via stacked W)
                    nc.tensor.matmul(
                        out=ewml[:, KUP:], lhsT=srcT[:], rhs=wallp[:],
                        start=True, stop=True)
                    # tensor_tensor may read only one PSUM operand: stage ew
                    ewsb = srcp.tile([P, KUP], f32, tag="ewsb")
                    nc.scalar.activation(
                        out=ewsb[:], in_=ewml[:, :KUP], func=ACT.Copy)
                    # msgs = ml * ew   (cols KU..KUP are exact zeros)
                    msg = msgp.tile([P, KUP], f32r)
                    nc.vector.tensor_mul(
                        out=msg[:], in0=ewml[:, KUP:], in1=ewsb[:])
                    # one-hot of dst within this node tile
                    oh = ohp.tile([P, P], f32r)
                    nc.gpsimd.tensor_scalar(
                        out=oh[:], in0=iota_r[:],
                        scalar1=dstv_all[:, t * L + j:t * L + j + 1],
                        scalar2=None, op0=ALU.is_equal)
                    # scatter-add into node accumulator
                    nc.tensor.matmul(
                        out=acc[:], lhsT=oh[:], rhs=msg[:],
                        start=(j == 0), stop=(j == L - 1))

                # ---- node-tile epilogue ----
                mean = epi.tile([P, KU], f32)
                nc.vector.tensor_scalar(
                    out=mean[:], in0=acc[:, :KU],
                    scalar1=recip_all[:, t:t + 1], scalar2=None, op0=ALU.mult)
                mta_ps = ptr.tile([96, P], f32, space="PSUM", tag="trps")
                nc.tensor.transpose(
                    out=mta_ps[:], in_=mean[:, 0:96], identity=ident_f[:])
                mta = epi.tile([96, P], f32, tag="mta")
                nc.scalar.activation(out=mta[:], in_=mta_ps[:], func=ACT.Copy)
                mtb_ps = ptr.tile([96, P], f32, space="PSUM", tag="trps")
                nc.tensor.transpose(
                    out=mtb_ps[:], in_=mean[:, 96:KU], identity=ident_f[:])
                mtb = epi.tile([96, P], f32, tag="mtb")
                nc.scalar.activation(out=mtb[:], in_=mtb_ps[:], func=ACT.Copy)
                fu = ph.tile([P, U], f32, space="PSUM", tag="hps")
                nc.tensor.matmul(out=fu[:], lhsT=mta[:], rhs=fwa[:],
                                 start=True, stop=False)
                nc.tensor.matmul(out=fu[:], lhsT=mtb[:], rhs=fwb[:],
                                 start=False, stop=False)
                nc.tensor.matmul(out=fu[:], lhsT=ones1[:], rhs=fbrow[:],
                                 start=False, stop=True)

                # group norm over 16 groups of 4
                fu3 = fu[:].rearrange("p (g d) -> p g d", g=G)
                sumg = epi.tile([P, G], f32, tag="sumg")
                nc.vector.tensor_reduce(
                    out=sumg[:], in_=fu3, axis=AX.X, op=ALU.add)
                mug = epi.tile([P, G], f32, tag="mug")
                nc.vector.tensor_scalar(
                    out=mug[:], in0=sumg[:], scalar1=1.0 / (U // G),
                    scalar2=None, op0=ALU.mult)
                xsq = epi.tile([P, U], f32, tag="xsq")
                nc.scalar.activation(out=xsq[:], in_=fu[:], func=ACT.Square)
                sqg = epi.tile([P, G], f32, tag="sqg")
                nc.vector.tensor_reduce(
                    out=sqg[:], in_=xsq[:].rearrange("p (g d) -> p g d", g=G),
                    axis=AX.X, op=ALU.add)
                varg = epi.tile([P, G], f32, tag="varg")
                # var = sq/4 - mu^2
                nc.vector.tensor_scalar(
                    out=varg[:], in0=sqg[:], scalar1=1.0 / (U // G),
                    scalar2=None, op0=ALU.mult)
                mu2 = epi.tile([P, G], f32, tag="mu2")
                nc.vector.tensor_mul(out=mu2[:], in0=mug[:], in1=mug[:])
                nc.vector.tensor_sub(out=varg[:], in0=varg[:], in1=mu2[:])
                stdg = epi.tile([P, G], f32, tag="stdg")
                nc.scalar.activation(out=stdg[:], in_=varg[:], func=ACT.Sqrt,
                                     bias=epsb[:, 0:1], scale=1.0)
                rstd = epi.tile([P, G], f32, tag="rstd")
                nc.vector.reciprocal(out=rstd[:], in_=stdg[:])

                xn = epi.tile([P, U], f32, tag="xn")
                xn3 = xn[:].rearrange("p (g d) -> p g d", g=G)
                mub = mug[:].unsqueeze(2).broadcast_to([P, G, U // G])
                rstdb = rstd[:].unsqueeze(2).broadcast_to([P, G, U // G])
                nc.vector.tensor_sub(out=xn3, in0=fu3, in1=mub)
                nc.vector.tensor_mul(out=xn3, in0=xn3, in1=rstdb)
                nc.vector.tensor_mul(out=xn[:], in0=xn[:], in1=gammab[:])
                nc.vector.tensor_add(out=xn[:], in0=xn[:], in1=betab[:])
                outf = epi.tile([P, U], f32, tag="outf")
                nc.scalar.activation(out=outf[:], in_=xn[:], func=ACT.Relu)
                nc.sync.dma_start(
                    out=d_out[t * P:t * P + rows, :], in_=outf[:rows, :])

    nc.compile()
    return nc


# ---------------------------------------------------------------- execution

_CACHE = {}


def _get_nc(cfg):
    key = (cfg.N, cfg.E, cfg.ncores, cfg.L)
    if key not in _CACHE:
        _CACHE[key] = build_nc(cfg)
    return _CACHE[key]


def run(cfg, inputs, trace=False):
    from concourse.bass_utils import run_bass_kernel_spmd
    L = cfg.L
    while True:
        try:
            in_maps = prepare_inputs(cfg, inputs)
            break
        except OverflowError as e:
            L = int(e.args[0]) + 1
            cfg = Cfg(cfg.N, cfg.E, cfg.ncores, L)
    nc = _get_nc(cfg)
    res = run_bass_kernel_spmd(nc, in_maps, list(range(cfg.ncores)),
                               trace=trace)
    out = np.concatenate([r["out"] for r in res.results], axis=0)
    return out.astype(np.float32), res


def kernel(**inputs):
    out, _ = run(FULL, inputs)
    return out
